# revision 1
# baseline (speedup 1.0000x reference)
"""Multi-head self-attention (B=4, N=2048, C=1024, H=16, D=64) on 8 NeuronCores.

Sharding: (batch, head-group) -> core.  Core i handles batch b = i // 2 and
heads hg = i % 2 (8 heads each).  Each core computes its 8 heads' attention and
a partial output projection; the host sums the two partials per batch element
and adds b_o.

Per-core device pipeline (all matmul inputs bf16, fp32 PSUM accumulation):
  xT [C, N] (x transposed on host)
  QT = (w_q.T @ x.T + b_q) stored [head-dims, N]   (d on partitions, head pair per 128)
  KT likewise;  V natural [N, head-dims] with a ones column per head (row sums)
  S^T[j, q] = K^T.T @ Q^T per head (keys on partitions)  ->  exp via ACT (scale 1/8)
  O^T[d, q] += Vpad.T @ P^T accumulated over j tiles; row 64 = softmax denominator
  normalize via reciprocal + ones-broadcast matmul, evict to OT [head-dims, N]
  out[q, :] = OT.T @ w_o  (partial; host adds pair + b_o)
"""

import sys
import numpy as np

sys.path.insert(0, "/opt/trn_rl_repo")

import ml_dtypes  # noqa: E402

B, N, C, H, D = 4, 2048, 1024, 16, 64
P = 128
NCORES = 8
HEADS_PER_CORE = H // 2  # 8
HD = HEADS_PER_CORE * D  # 512 head-dims per core

_cache = {}


def _build_nc(n=N, c=C, heads=HEADS_PER_CORE, d=D, qcn=512, num_devices=NCORES,
              dt_name="bfloat16", debug_dump=False, repeats=1, npro_v=2,
              mm_bufs=2, sp_bufs=2, pt_bufs=6, proj_pair=True,
              oproj_pair=None):
    import concourse.bacc as bacc
    import concourse.tile as tile
    import concourse.mybir as mybir

    dt = getattr(mybir.dt, dt_name)
    f32 = mybir.dt.float32
    f32r = mybir.dt.float32r
    add_op = mybir.AluOpType.add
    Exp = mybir.ActivationFunctionType.Exp

    hd = heads * d
    CT = c // P            # qkv contraction tiles
    MT = hd // P           # head-pair tiles (2 heads per tile)
    QC = n // qcn          # query chunks
    NT = n // P            # sequence tiles (key/j tiles)
    OCN = min(512, c)      # oproj output column chunk
    OC = c // OCN
    KO = hd // P           # oproj contraction tiles
    scale = float(d) ** -0.5
    if oproj_pair is None:
        oproj_pair = proj_pair
    assert d == 64 and MT * P == hd and CT * P == c

    nc = bacc.Bacc("TRN2", target_bir_lowering=False, debug=False,
                   num_devices=num_devices)

    xT_d = nc.declare_dram_parameter("xT", [c, n], dt, isOutput=False)
    wq_d = nc.declare_dram_parameter("wq", [c, hd], dt, isOutput=False)
    wk_d = nc.declare_dram_parameter("wk", [c, hd], dt, isOutput=False)
    wv_d = nc.declare_dram_parameter("wv", [c, hd], dt, isOutput=False)
    wo_d = nc.declare_dram_parameter("wo", [hd, c], dt, isOutput=False)
    bq_d = nc.declare_dram_parameter("bq", [MT, P], f32, isOutput=False)
    bk_d = nc.declare_dram_parameter("bk", [MT, P], f32, isOutput=False)
    bv_d = nc.declare_dram_parameter("bv", [P, hd], f32, isOutput=False)
    out_d = nc.declare_dram_parameter("out", [n, c], f32, isOutput=True)
    if debug_dump:
        dbg_qt = nc.declare_dram_parameter("dbg_qt", [P, MT, n], dt, isOutput=True)
        dbg_kt = nc.declare_dram_parameter("dbg_kt", [P, MT, n], dt, isOutput=True)
        dbg_vp = nc.declare_dram_parameter("dbg_vp", [P, NT, heads * (d + 1)], dt, isOutput=True)
        dbg_ot = nc.declare_dram_parameter("dbg_ot", [P, KO, n], dt, isOutput=True)
        dbg_bvb = nc.declare_dram_parameter("dbg_bvb", [P, hd], f32, isOutput=True)

    with tile.TileContext(nc) as tc:
        with tc.tile_pool(name="singles", bufs=1) as singles, \
             tc.tile_pool(name="pt_pool", bufs=pt_bufs) as pt_pool, \
             tc.tile_pool(name="norm_pool", bufs=4) as norm_pool, \
             tc.tile_pool(name="evict_pool", bufs=4) as evict_pool, \
             tc.tile_pool(name="ps_mm", bufs=mm_bufs, space="PSUM") as ps_mm, \
             tc.tile_pool(name="ps_sp", bufs=sp_bufs, space="PSUM") as ps_sp, \
             tc.tile_pool(name="ps_o", bufs=1, space="PSUM") as ps_o:

            # ---- resident tensors -------------------------------------
            xt = singles.tile([P, CT, n], dt)
            wqt = singles.tile([P, CT, hd], dt)
            wkt = singles.tile([P, CT, hd], dt)
            wvt = singles.tile([P, CT, hd], dt)
            wot = singles.tile([P, KO, c], dt)
            bqt = singles.tile([P, MT], f32)
            bkt = singles.tile([P, MT], f32)
            bvb = singles.tile([P, hd], f32)
            qt_t = singles.tile([P, MT, n], dt)
            kt_t = singles.tile([P, MT, n], dt)
            vpad = singles.tile([P, NT, heads * (d + 1)], dt)
            ot_t = singles.tile([P, KO, n], dt)
            ones1_f = singles.tile([1, 64], f32)

            for ct in range(CT):
                cs = slice(ct * P, (ct + 1) * P)
                nc.sync.dma_start(out=xt[:, ct, :], in_=xT_d[cs, :])
                nc.sync.dma_start(out=wkt[:, ct, :], in_=wk_d[cs, :])
                nc.sync.dma_start(out=wvt[:, ct, :], in_=wv_d[cs, :])
                nc.sync.dma_start(out=wqt[:, ct, :], in_=wq_d[cs, :])
            for ko in range(KO):
                nc.sync.dma_start(out=wot[:, ko, :], in_=wo_d[ko * P:(ko + 1) * P, :])
            nc.sync.dma_start(out=bqt, in_=bq_d[:].rearrange("t p -> p t"))
            nc.sync.dma_start(out=bkt, in_=bk_d[:].rearrange("t p -> p t"))
            nc.sync.dma_start(out=bvb, in_=bv_d[:, :])
            nc.vector.memset(ones1_f, 1.0)
            ones1 = ones1_f.bitcast(f32r)
            nc.vector.memset(vpad, 1.0)  # ones columns; V parts overwritten
            for _rep in range(repeats):
                # ---- chunk emitters (each emits one PSUM group + evict) ----
                def _mm_ops(n_steps, per, alloc, mm_step, fin):
                    """Micro-ops for one PSUM group: `per` matmul steps per op,
                    then a finishing op. State holds the lazily-made tile."""
                    state = {}
                    ops = []
                    for s0 in range(0, n_steps, per):
                        def op(s0=s0):
                            if "t" not in state:
                                state["t"] = alloc()
                            for s in range(s0, min(s0 + per, n_steps)):
                                mm_step(state["t"], s)
                        ops.append(op)
                    ops.append(lambda: fin(state["t"]))
                    return ops

                def v_chunk(nt):
                    def alloc():
                        return ps_mm.tile([P, hd], f32, tag="mm", name=f"psv{nt}")

                    def mm(t, ct):
                        nc.tensor.matmul(t, xt[:, ct, nt * P:(nt + 1) * P],
                                         wvt[:, ct, :],
                                         start=(ct == 0), stop=(ct == CT - 1))

                    def fin(t):
                        vtgt = vpad[:, nt, :].rearrange("p (h e) -> p h e", e=d + 1)[:, :, :d]
                        nc.vector.tensor_add(
                            vtgt,
                            t.rearrange("p (h e) -> p h e", e=d),
                            bvb.rearrange("p (h e) -> p h e", e=d),
                        )
                    return _mm_ops(CT, 2, alloc, mm, fin)

                def proj_chunk(w_t, b_t, dst, mt, qc):
                    qs = slice(qc * qcn, (qc + 1) * qcn)

                    def alloc():
                        return ps_mm.tile([P, qcn], f32, tag="mm", name=f"psp{mt}_{qc}")

                    def mm(t, ct):
                        nc.tensor.matmul(t, w_t[:, ct, mt * P:(mt + 1) * P],
                                         xt[:, ct, qs],
                                         start=(ct == 0), stop=(ct == CT - 1))

                    def fin(t):
                        nc.vector.tensor_scalar(
                            out=dst[:, mt, qs], in0=t,
                            scalar1=b_t[:, mt:mt + 1], scalar2=None, op0=add_op)
                    return _mm_ops(CT, 2, alloc, mm, fin)

                def proj_chunk_pair(w_t, b_t, dst, mt, qc0, qc1):
                    # two q-chunks per emission: consecutive matmuls share one
                    # lhsT (halves projection weight loads); needs both mm bufs
                    qs0 = slice(qc0 * qcn, (qc0 + 1) * qcn)
                    qs1 = slice(qc1 * qcn, (qc1 + 1) * qcn)
                    state = {}
                    ops = []

                    def mk(ct):
                        def op():
                            if "a" not in state:
                                state["a"] = ps_mm.tile([P, qcn], f32, tag="mm",
                                                        name=f"pspa{mt}_{qc0}")
                                state["b"] = ps_mm.tile([P, qcn], f32, tag="mm",
                                                        name=f"pspb{mt}_{qc1}")
                            lhs = w_t[:, ct, mt * P:(mt + 1) * P]
                            nc.tensor.matmul(state["a"], lhs, xt[:, ct, qs0],
                                             start=(ct == 0), stop=(ct == CT - 1))
                            nc.tensor.matmul(state["b"], lhs, xt[:, ct, qs1],
                                             start=(ct == 0), stop=(ct == CT - 1))
                        return op
                    for ct in range(CT):
                        ops.append(mk(ct))

                    def fin():
                        nc.vector.tensor_scalar(
                            out=dst[:, mt, qs0], in0=state["a"],
                            scalar1=b_t[:, mt:mt + 1], scalar2=None, op0=add_op)
                        nc.vector.tensor_scalar(
                            out=dst[:, mt, qs1], in0=state["b"],
                            scalar1=b_t[:, mt:mt + 1], scalar2=None, op0=add_op)
                    ops.append(fin)
                    return ops

                def oproj_chunk(qt_i, oc):
                    ts_ = slice(qt_i * P, (qt_i + 1) * P)
                    ocs = slice(oc * OCN, (oc + 1) * OCN)

                    def alloc():
                        return ps_mm.tile([P, OCN], f32, tag="mm", name=f"pso{qt_i}_{oc}")

                    def mm(t, ko):
                        nc.tensor.matmul(t, ot_t[:, ko, ts_], wot[:, ko, ocs],
                                         start=(ko == 0), stop=(ko == KO - 1))

                    def fin(t):
                        st = evict_pool.tile([P, OCN], f32, tag="st", name=f"st{qt_i}_{oc}")
                        nc.vector.tensor_copy(st, t)
                        nc.sync.dma_start(out=out_d[ts_, ocs], in_=st)
                    return _mm_ops(KO, 2, alloc, mm, fin)

                def oproj_chunk_pair(qt_i):
                    # both output-column chunks per lhsT (one weight load
                    # feeds two open psum groups, like proj_chunk_pair)
                    ts_ = slice(qt_i * P, (qt_i + 1) * P)
                    state = {}
                    ops = []

                    def mk(ko):
                        def op():
                            if "a" not in state:
                                state["a"] = ps_mm.tile([P, OCN], f32, tag="mm",
                                                        name=f"psoa{qt_i}")
                                state["b"] = ps_mm.tile([P, OCN], f32, tag="mm",
                                                        name=f"psob{qt_i}")
                            lhs = ot_t[:, ko, ts_]
                            nc.tensor.matmul(state["a"], lhs, wot[:, ko, 0:OCN],
                                             start=(ko == 0), stop=(ko == KO - 1))
                            nc.tensor.matmul(state["b"], lhs, wot[:, ko, OCN:2 * OCN],
                                             start=(ko == 0), stop=(ko == KO - 1))
                        return op
                    for ko in range(KO):
                        ops.append(mk(ko))

                    def fin():
                        for key, ocs in (("a", slice(0, OCN)),
                                         ("b", slice(OCN, 2 * OCN))):
                            st = evict_pool.tile([P, OCN], f32, tag="st",
                                                 name=f"st{qt_i}_{key}")
                            nc.vector.tensor_copy(st, state[key])
                            nc.sync.dma_start(out=out_d[ts_, ocs], in_=st)
                    ops.append(fin)
                    return ops

                def oproj_chunks(qc):
                    if oproj_pair and OC == 2:
                        return [oproj_chunk_pair(qt_i)
                                for qt_i in range(qc * (qcn // P), (qc + 1) * (qcn // P))]
                    return [oproj_chunk(qt_i, oc)
                            for qt_i in range(qc * (qcn // P), (qc + 1) * (qcn // P))
                            for oc in range(OC)]

                # ---- fill queues: who runs inside which attention loop ----
                # KT(mt+1) must be fully done before attention(mt+1, 0);
                # QT(mt+1, qc) before attention(mt+1, qc); V(nt) before PV jt=nt
                NPRO_V = min(npro_v, NT)  # V chunks emitted in the prologue
                fillq = {(mt_, qc_): [] for mt_ in range(MT) for qc_ in range(QC)}
                for nt in range(NPRO_V, NT):
                    fillq[(0, 0)] += v_chunk(nt)
                # remaining first-pair QT chunks: QT0(q) emitted during (0, q-1)
                if proj_pair and QC == 4:
                    fillq[(0, 0)] += proj_chunk(wqt, bqt, qt_t, 0, 1)
                    fillq[(0, 1)] += proj_chunk_pair(wqt, bqt, qt_t, 0, 2, 3)
                else:
                    for q in range(1, QC):
                        fillq[(0, q - 1)] += proj_chunk(wqt, bqt, qt_t, 0, q)
                # projections for pair mt+1 spread over pair mt's qc loops
                # (KT chunks first: KT(mt+1) must be complete before
                #  attention(mt+1, 0); QT(mt+1, q) before attention(mt+1, q))
                for mt_ in range(MT - 1):
                    nxt = mt_ + 1
                    if proj_pair and QC % 2 == 0:
                        chunks = [proj_chunk_pair(wkt, bkt, kt_t, nxt, q, q + 1)
                                  for q in range(0, QC, 2)] + \
                                 [proj_chunk_pair(wqt, bqt, qt_t, nxt, q, q + 1)
                                  for q in range(0, QC, 2)]
                    else:
                        chunks = [proj_chunk(wkt, bkt, kt_t, nxt, q) for q in range(QC)] + \
                                 [proj_chunk(wqt, bqt, qt_t, nxt, q) for q in range(QC)]
                    per = (len(chunks) + QC - 1) // QC
                    for i, chk in enumerate(chunks):
                        fillq[(mt_, min(i // per, QC - 1))] += chk
                for qc_ in range(1, QC):
                    for chk in oproj_chunks(qc_ - 1):
                        fillq[(MT - 1, qc_)] += chk

                # ---- prologue: minimum work before attention(0, 0) ---------
                if proj_pair and QC % 2 == 0:
                    for q in range(0, QC, 2):
                        for op in proj_chunk_pair(wkt, bkt, kt_t, 0, q, q + 1):
                            op()
                else:
                    for q in range(QC):
                        for op in proj_chunk(wkt, bkt, kt_t, 0, q):
                            op()
                for op in proj_chunk(wqt, bqt, qt_t, 0, 0):
                    op()
                for nt in range(NPRO_V):
                    for op in v_chunk(nt):
                        op()

                # ---- attention, software-pipelined across head pairs -------
                for mt in range(MT):
                    for qc in range(QC):
                        qs = slice(qc * qcn, (qc + 1) * qcn)
                        fill = fillq[(mt, qc)]
                        fill0, popped = len(fill), 0

                        po = ps_o.tile([65, 2, qcn], f32, tag="po", name=f"po{qc}_{mt}")
                        for jt in range(NT):
                            # fill paced evenly at the top of each slot: ops
                            # must EMIT before consumers (Tile deps are
                            # established at emission time)
                            want = ((jt + 1) * fill0 + NT - 1) // NT
                            while popped < want and fill:
                                fill.pop(0)()
                                popped += 1
                            js = slice(jt * P, (jt + 1) * P)
                            psS = ps_sp.tile([P, 2, qcn], f32, tag="sp", name=f"psS{jt}")
                            nc.tensor.matmul(psS[:, 0, :], kt_t[0:64, mt, js],
                                             qt_t[0:64, mt, qs], start=True, stop=True)
                            nc.tensor.matmul(psS[:, 1, :], kt_t[64:128, mt, js],
                                             qt_t[64:128, mt, qs], start=True, stop=True)
                            ptp = pt_pool.tile([P, 2, qcn], dt, tag="pt", name=f"ptp{jt}")
                            nc.scalar.activation(ptp, psS, Exp, scale=scale)
                            hA, hB = 2 * mt, 2 * mt + 1
                            nc.tensor.matmul(po[:, 0, :], vpad[:, jt, hA * (d + 1):(hA + 1) * (d + 1)],
                                             ptp[:, 0, :], start=(jt == 0), stop=(jt == NT - 1),
                                             skip_group_check=True)
                            nc.tensor.matmul(po[:, 1, :], vpad[:, jt, hB * (d + 1):(hB + 1) * (d + 1)],
                                             ptp[:, 1, :], start=(jt == 0), stop=(jt == NT - 1),
                                             skip_group_check=True)
                        # copy po -> SBUF in one op so the PSUM accumulator
                        # frees immediately; normalize off the critical path
                        ocp = norm_pool.tile([65, 2, qcn], f32, tag="ocp", name="ocp")
                        nc.vector.tensor_copy(ocp, po)
                        rcp = norm_pool.tile([1, 2, qcn], f32r, tag="rcp", name="rcp")
                        with nc.allow_low_precision(reason="f32r is 4-byte"):
                            nc.vector.reciprocal(rcp, ocp[64:65, :, :])
                        for side in (0, 1):
                            pb = ps_mm.tile([64, qcn], f32, tag="mm", name=f"pb{side}")
                            nc.tensor.matmul(pb, ones1, rcp[:, side, :],
                                             start=True, stop=True)
                            if side == 0:
                                nc.vector.tensor_mul(ot_t[0:64, mt, qs], ocp[0:64, 0, :], pb)
                            else:
                                tmpB = norm_pool.tile([64, qcn], dt, tag="tmpB", name="tmpB")
                                nc.vector.tensor_mul(tmpB, ocp[0:64, 1, :], pb)
                                nc.sync.dma_start(out=ot_t[64:128, mt, qs], in_=tmpB)
                        for op in fill:
                            op()
                # last oproj chunk after the final attention group
                for chk in oproj_chunks(QC - 1):
                    for op in chk:
                        op()

            if debug_dump:
                nc.sync.dma_start(out=dbg_qt[:], in_=qt_t)
                nc.sync.dma_start(out=dbg_kt[:], in_=kt_t)
                nc.sync.dma_start(out=dbg_vp[:], in_=vpad)
                nc.sync.dma_start(out=dbg_ot[:], in_=ot_t)
                nc.sync.dma_start(out=dbg_bvb[:], in_=bvb)

    nc.compile()
    return nc


def _get_runner():
    """Build nc once and return a cached callable in_maps -> list of out dicts.

    Replicates run_bass_kernel_spmd's axon/PJRT path (bass2jax) but keeps the
    jitted executable cached across kernel() invocations so the NEFF is
    compiled exactly once per process.
    """
    if "runner" in _cache:
        return _cache["runner"]

    import jax
    from jax.experimental.shard_map import shard_map
    from jax.sharding import Mesh, PartitionSpec
    import concourse.mybir as mybir
    from concourse.bass2jax import (_bass_exec_p, install_neuronx_cc_hook,
                                    partition_id_tensor)

    nc = _build_nc()
    _cache["nc"] = nc
    install_neuronx_cc_hook()

    partition_name = (nc.partition_id_tensor.name
                      if nc.partition_id_tensor else None)
    in_names, out_names, out_avals, zero_outs = [], [], [], []
    for alloc in nc.m.functions[0].allocations:
        if not isinstance(alloc, mybir.MemoryLocationSet):
            continue
        name = alloc.memorylocations[0].name
        if alloc.kind == "ExternalInput":
            if name != partition_name:
                in_names.append(name)
        elif alloc.kind == "ExternalOutput":
            out_names.append(name)
            shape = tuple(alloc.tensor_shape)
            np_dt = mybir.dt.np(alloc.dtype)
            out_avals.append(jax.core.ShapedArray(shape, np_dt))
            zero_outs.append(np.zeros(shape, np_dt))
    n_params = len(in_names)
    n_outs = len(out_avals)
    all_in_names = list(in_names) + list(out_names)
    if partition_name is not None:
        all_in_names.append(partition_name)

    def _body(*args):
        operands = list(args)
        if partition_name is not None:
            operands.append(partition_id_tensor())
        outs = _bass_exec_p.bind(
            *operands,
            out_avals=tuple(out_avals),
            in_names=tuple(all_in_names),
            out_names=tuple(out_names),
            lowering_input_output_aliases=(),
            sim_require_finite=True,
            sim_require_nnan=True,
            nc=nc,
        )
        return tuple(outs)

    devices = jax.devices()[:NCORES]
    assert len(devices) == NCORES, f"need {NCORES} cores, have {len(jax.devices())}"
    mesh = Mesh(np.asarray(devices), ("core",))
    in_specs = (PartitionSpec("core"),) * (n_params + n_outs)
    out_specs = (PartitionSpec("core"),) * n_outs
    sharded = jax.jit(
        shard_map(_body, mesh=mesh, in_specs=in_specs, out_specs=out_specs,
                  check_rep=False),
        donate_argnums=tuple(range(n_params, n_params + n_outs)),
        keep_unused=True,
    )

    def runner(in_maps):
        per_core = [[np.asarray(m[name]) for name in in_names] for m in in_maps]
        concat_in = [
            np.concatenate([per_core[cr][i] for cr in range(NCORES)], axis=0)
            for i in range(n_params)
        ] + [
            np.concatenate([z] * NCORES, axis=0) for z in zero_outs
        ]
        out_arrs = sharded(*concat_in)
        results = []
        for cr in range(NCORES):
            res = {}
            for i, name in enumerate(out_names):
                arr = np.asarray(out_arrs[i])
                rows = arr.shape[0] // NCORES
                res[name] = arr[cr * rows:(cr + 1) * rows]
            results.append(res)
        return results

    _cache["runner"] = runner
    _cache["meta"] = (in_names, out_names, out_avals, zero_outs, partition_name)
    return runner


def make_in_maps(x, w_q, b_q, w_k, b_k, w_v, b_v, w_o, b_o):
    bf16 = ml_dtypes.bfloat16
    in_maps = []
    for core in range(NCORES):
        b = core // 2
        hs = (core % 2) * HD
        in_maps.append({
            "xT": np.ascontiguousarray(x[b].T).astype(bf16),
            "wq": np.ascontiguousarray(w_q[:, hs:hs + HD]).astype(bf16),
            "wk": np.ascontiguousarray(w_k[:, hs:hs + HD]).astype(bf16),
            "wv": np.ascontiguousarray(w_v[:, hs:hs + HD]).astype(bf16),
            "wo": np.ascontiguousarray(w_o[hs:hs + HD, :]).astype(bf16),
            "bq": np.ascontiguousarray(b_q[hs:hs + HD].reshape(-1, P)).astype(np.float32),
            "bk": np.ascontiguousarray(b_k[hs:hs + HD].reshape(-1, P)).astype(np.float32),
            "bv": np.ascontiguousarray(np.broadcast_to(
                b_v[hs:hs + HD].astype(np.float32), (P, HD))),
        })
    return in_maps


def kernel(x, w_q, b_q, w_k, b_k, w_v, b_v, w_o, b_o):
    x, w_q, b_q, w_k, b_k, w_v, b_v, w_o, b_o = (
        np.asarray(t, dtype=np.float32)
        for t in (x, w_q, b_q, w_k, b_k, w_v, b_v, w_o, b_o))
    runner = _get_runner()
    in_maps = make_in_maps(x, w_q, b_q, w_k, b_k, w_v, b_v, w_o, b_o)
    results = runner(in_maps)
    out = np.empty((B, N, C), np.float32)
    bo = np.asarray(b_o, dtype=np.float32)
    for b in range(B):
        out[b] = results[2 * b]["out"] + results[2 * b + 1]["out"] + bo
    return out

